# revision 5
# baseline (speedup 1.0000x reference)
"""AttentionBlock kernel v2 for 8x Trainium2 NeuronCores.

Data-parallel over batch: core b computes batch element b end-to-end.
Per core: x [512, 1024] -> GroupNorm(32) -> qkv -> 8-head attention -> proj
+ residual -> y [512, 1024].

v2 structure (from HW microbenchmarks + cost model):
  - fp8e4m3 weights (scaled x32, folded back in the bias step) and fp8
    activations for v/proj; q/k kept bf16. Halves weight DMA 4x and
    enables DoubleRow (K=256) matmuls at 0.5 cyc/row for qkv/AV/proj.
  - exp is the hard floor (ACT fixed-rate ~1.1us per [128,1024] tile,
    64 tiles): the loop is ACT-bound, PE has slack, so all projections
    run outside the loop, no filler machinery.
  - PSUM: score tag bufs=2 (4 banks) + one [128,1024] AV accumulator
    (A-head rows 0:64, B-head rows 64:128 via DR out-partition offset)
    + aux tag (pd denominators / prn broadcast) = 8 banks exactly.
  - softmax denominator rides tiny ones-lhsT DR matmuls into pd;
    per-pair normalization (recip + PE broadcast + one DVE mult into
    fp8 ar) overlaps the next pair's ACT-bound phase.
  - residual uses the resident fp32 x (no DRAM reload); xpb = x + proj
    bias precomputed on DVE during the loop.
"""

import sys

sys.path.insert(0, "/opt/trn_rl_repo")

import numpy as np

B, C, T = 8, 512, 1024
NH, CH = 8, 64
NG, GS = 32, 16
EPS = 1e-5
N_CORES = 8
CT = C // 128  # channel tiles (4)
TB = T // 128  # s blocks (8)
NP = NH // 2  # head pairs (4)
SW = 32.0  # weight scale (fp8 subnormal avoidance)
SA = 4.0  # ar scale

_CACHE = {}


def _install_compile_patches():
    from concourse import bass_utils

    if getattr(bass_utils, "_ldw_opt_patched", False):
        return
    orig = bass_utils.run_command

    def patched(cmd, **kw):
        cmd = [
            c.replace("--enable-ldw-opt=false", "--enable-ldw-opt=true")
            if isinstance(c, str)
            else c
            for c in cmd
        ]
        return orig(cmd, **kw)

    bass_utils.run_command = patched
    bass_utils._ldw_opt_patched = True


def _install_tile_drain_patch(tile_mod, vector_clock_mod, bass_rust_mod):
    """Split TileContext's exit-drain waits over multiple SP nops (CTRL
    instructions accept a single sync wait on this walrus)."""
    ScopedClock = vector_clock_mod.ScopedClock

    def _patched(self, tick_clock, wait_clock):
        nc = self.nc
        probe = nc.sync.nop(nofuse=True)
        wait_clock.add_sem_waits(
            probe.ins, ScopedClock({None: tick_clock.global_clock})
        )
        waits = list(probe.ins.sync_info.on_wait) if probe.ins.sync_info else []
        probe.ins.sync_info = bass_rust_mod.SyncInfo(
            on_wait=waits[:1], on_update=[]
        )
        for w in waits[1:]:
            extra = nc.sync.nop(nofuse=True)
            extra.ins.sync_info = bass_rust_mod.SyncInfo(
                on_wait=[w], on_update=[]
            )
        nc.sync.drain()
        nc.all_engine_barrier()
        assert self.sems is not None
        popped = nc._tile_sem_poison_stack.pop()
        assert popped is self._sem_poison
        nc.clear_and_free_semaphores(list(self.sems.allocated().values()))
        nc.all_engine_barrier()

    tile_mod.TileContext._drain_and_barrier = _patched


def _split_excess_waits(nc, mybir, bass_rust, cap=1):
    cnt = 0
    for fn in nc.m.functions:
        for bb in fn.blocks:
            il = bb.instructions
            new_list = []
            for ins in il:
                si = ins.sync_info
                waits = list(si.on_wait) if si and si.on_wait else []
                if len(waits) > cap:
                    for w in waits[:-cap]:
                        cnt += 1
                        new_list.append(
                            mybir.InstNoOp(
                                name=f"waitsplit-{cnt}",
                                engine=ins.engine,
                                ins=[],
                                outs=[],
                                sync_info=bass_rust.SyncInfo(
                                    on_wait=[w], on_update=[]
                                ),
                            )
                        )
                    ins.sync_info = bass_rust.SyncInfo(
                        on_wait=waits[-cap:],
                        on_update=list(si.on_update) if si.on_update else [],
                    )
                new_list.append(ins)
            il[:] = new_list
    return cnt


def build_nc(loop_n=None, debug=False):
    from contextlib import nullcontext
    from concourse import bass, mybir, tile
    from concourse import vector_clock
    import bass_rust

    _install_tile_drain_patch(tile, vector_clock, bass_rust)

    f32 = mybir.dt.float32
    f32r = mybir.dt.float32r
    bf16 = mybir.dt.bfloat16
    fp8 = mybir.dt.float8e4
    u8 = mybir.dt.uint8
    AL = mybir.AluOpType
    AF = mybir.ActivationFunctionType
    DR = mybir.MatmulPerfMode.DoubleRow

    nc = bass.Bass(num_devices=N_CORES)

    # --- I/O (fp8 weights travel as uint8, bitcast on DMA) ---
    x = nc.declare_dram_parameter("x", [C, T], f32, isOutput=False)
    wq8 = nc.declare_dram_parameter("wq8", [C, C], u8, isOutput=False)  # [c,o]
    wk8 = nc.declare_dram_parameter("wk8", [C, C], u8, isOutput=False)
    wv8 = nc.declare_dram_parameter("wv8", [C, C], u8, isOutput=False)
    pw8 = nc.declare_dram_parameter("pw8", [C, C], u8, isOutput=False)
    bq = nc.declare_dram_parameter("bq", [C], f32, isOutput=False)  # pre-scaled
    bk = nc.declare_dram_parameter("bk", [C], f32, isOutput=False)
    pb = nc.declare_dram_parameter("pb", [C], f32, isOutput=False)
    nsc = nc.declare_dram_parameter("nsc", [C], f32, isOutput=False)
    nbi = nc.declare_dram_parameter("nbi", [C], f32, isOutput=False)
    gmap = nc.declare_dram_parameter("gmap", [C, NG], f32, isOutput=False)
    emap = nc.declare_dram_parameter("emap", [NG, C], f32, isOutput=False)
    ones8 = nc.declare_dram_parameter("ones8", [128, 128], u8, isOutput=False)
    y = nc.declare_dram_parameter("y", [C, T], f32, isOutput=True)
    if debug:
        dbg_xt8 = nc.declare_dram_parameter("dbg_xt8", [C, T], u8, isOutput=True)
        dbg_q = nc.declare_dram_parameter("dbg_q", [C, T], bf16, isOutput=True)
        dbg_k = nc.declare_dram_parameter("dbg_k", [C, T], bf16, isOutput=True)
        dbg_v = nc.declare_dram_parameter("dbg_v", [128, TB * C], u8, isOutput=True)
        dbg_ar = nc.declare_dram_parameter("dbg_ar", [C, T], u8, isOutput=True)
        dbg_pd = nc.declare_dram_parameter("dbg_pd", [NP * 128, T], f32, isOutput=True)

    with tile.TileContext(nc) as tc:
        with tc.tile_pool(name="persist", bufs=1) as pp:
            # --- persistent SBUF ---
            xt = pp.tile([128, CT * T], f32, name="xt")
            xpb = pp.tile([128, CT * T], f32, name="xpb")
            xt8 = pp.tile([128, CT * T], fp8, name="xt8")
            wq_sb = pp.tile([128, CT * C], fp8, name="wq_sb")
            wk_sb = pp.tile([128, CT * C], fp8, name="wk_sb")
            wv_sb = pp.tile([128, CT * C], fp8, name="wv_sb")
            pw_sb = pp.tile([128, CT * C], fp8, name="pw_sb")
            q16 = pp.tile([128, CT * T], bf16, name="q16")
            k16 = pp.tile([128, CT * T], bf16, name="k16")
            vta8 = pp.tile([128, TB * C], fp8, name="vta8")
            ar8 = pp.tile([128, CT * T], fp8, name="ar8")
            bq_sb = pp.tile([128, CT], f32, name="bq_sb")
            bk_sb = pp.tile([128, CT], f32, name="bk_sb")
            pb_sb = pp.tile([128, CT], f32, name="pb_sb")
            scl_sb = pp.tile([128, CT], f32, name="scl_sb")
            bia_sb = pp.tile([128, CT], f32, name="bia_sb")
            gmap_sb = pp.tile([128, CT * NG], f32, name="gmap_sb")
            emap_sb = pp.tile([NG, C], f32, name="emap_sb")
            ones8_sb = pp.tile([128, 128], fp8, name="ones8_sb")

            loop_cm = tc.For_i(0, loop_n, 1) if loop_n else nullcontext()

            # --- loop-invariant loads.  Queue plan (x must land ASAP, its
            # queue order is behind the loop-invariant weights):
            #   sync:   wq (1 desc), then per-iteration x halves
            #   scalar: wk (1 desc), then x halves
            #   SWDGE:  wv, consts, pw (wv needed ~12us, pw only at stage 4)
            nc.sync.dma_start(
                out=wq_sb[:, :].rearrange("p (j c) -> p j c", j=CT).bitcast(u8),
                in_=wq8[:, :].rearrange("(j p) c -> p j c", j=CT),
            )
            nc.scalar.dma_start(
                out=wk_sb[:, :].rearrange("p (j c) -> p j c", j=CT).bitcast(u8),
                in_=wk8[:, :].rearrange("(j p) c -> p j c", j=CT),
            )
            nc.gpsimd.dma_start(
                out=wv_sb[:, :].rearrange("p (j c) -> p j c", j=CT).bitcast(u8),
                in_=wv8[:, :].rearrange("(j p) c -> p j c", j=CT),
            )
            for dst, srcp in (
                (bq_sb, bq), (bk_sb, bk), (pb_sb, pb),
                (scl_sb, nsc), (bia_sb, nbi),
            ):
                nc.gpsimd.dma_start(
                    out=dst[:, :], in_=srcp[:].rearrange("(j p) -> p j", j=CT)
                )
            nc.gpsimd.dma_start(
                out=gmap_sb[:, :].rearrange("p (j g) -> p j g", j=CT),
                in_=gmap[:, :].rearrange("(j p) g -> p j g", j=CT),
            )
            nc.gpsimd.dma_start(out=emap_sb[:, :], in_=emap[:, :])
            nc.gpsimd.dma_start(
                out=ones8_sb[:, :].bitcast(u8), in_=ones8[:, :]
            )
            nc.gpsimd.dma_start(
                out=pw_sb[:, :].rearrange("p (j c) -> p j c", j=CT).bitcast(u8),
                in_=pw8[:, :].rearrange("(j p) c -> p j c", j=CT),
            )

            loop_cm.__enter__()
            dma2 = nc.scalar
            for j in range(CT):
                # loop mode: x owns the sync queue exclusively so the next
                # iteration's x-load prefetches during this iteration's
                # attention phase instead of queueing behind y-out
                eng = nc.sync if (loop_n or j % 2 == 0) else dma2
                eng.dma_start(
                    out=xt[:, j * T : (j + 1) * T],
                    in_=x[j * 128 : (j + 1) * 128, :],
                )

            # =========== Stage 1: GroupNorm -> xt8 (fp8) ===========
            with (
                tc.tile_pool(name="s1", bufs=1) as s1,
                tc.tile_pool(name="s1p", bufs=1, space="PSUM") as s1p,
            ):
                stats2 = s1.tile([128, 2 * CT], f32, name="stats2")
                for j in range(CT):
                    xtj = xt[:, j * T : (j + 1) * T]
                    nc.vector.tensor_reduce(
                        out=stats2[:, 2 * j : 2 * j + 1],
                        in_=xtj,
                        axis=mybir.AxisListType.X,
                        op=AL.add,
                    )
                    scr = s1.tile([128, T], f32, name="sq_scr", tag="sq", bufs=2)
                    nc.scalar.activation(
                        out=scr,
                        in_=xtj,
                        func=AF.Square,
                        accum_out=stats2[:, 2 * j + 1 : 2 * j + 2],
                    )
                pst = s1p.tile([NG, 2], f32, name="pst")
                for j in range(CT):
                    nc.tensor.matmul(
                        pst[:, :],
                        lhsT=gmap_sb[:, j * NG : (j + 1) * NG],
                        rhs=stats2[:, 2 * j : 2 * j + 2],
                        start=(j == 0),
                        stop=(j == CT - 1),
                    )
                grp = s1.tile([NG, 8], f32, name="grp")
                inv_n = 1.0 / (GS * T)
                nc.vector.tensor_scalar(
                    out=grp[:, 0:1], in0=pst[:, 0:1],
                    scalar1=inv_n, scalar2=None, op0=AL.mult,
                )
                nc.vector.tensor_scalar(
                    out=grp[:, 2:3], in0=pst[:, 1:2],
                    scalar1=inv_n, scalar2=None, op0=AL.mult,
                )
                nc.vector.tensor_tensor(
                    out=grp[:, 3:4], in0=grp[:, 0:1], in1=grp[:, 0:1], op=AL.mult
                )
                nc.vector.tensor_tensor(
                    out=grp[:, 2:3], in0=grp[:, 2:3], in1=grp[:, 3:4],
                    op=AL.subtract,
                )
                nc.vector.tensor_scalar(
                    out=grp[:, 2:3], in0=grp[:, 2:3],
                    scalar1=EPS, scalar2=None, op0=AL.add,
                )
                nc.scalar.activation(
                    out=grp[:, 3:4], in_=grp[:, 2:3], func=AF.Sqrt, bias=0.0
                )
                nc.vector.reciprocal(out=grp[:, 1:2], in_=grp[:, 3:4])

                ab = s1.tile([128, 2 * CT], f32, name="ab")
                for j in range(CT):
                    ppc = s1p.tile([128, 2], f32, name="ppc", tag="ppc", bufs=2)
                    nc.tensor.matmul(
                        ppc[:, :],
                        lhsT=emap_sb[:, j * 128 : (j + 1) * 128],
                        rhs=grp[:, 0:2],
                        start=True,
                        stop=True,
                    )
                    aj = ab[:, 2 * j : 2 * j + 1]
                    bj = ab[:, 2 * j + 1 : 2 * j + 2]
                    nc.vector.tensor_tensor(
                        out=aj, in0=ppc[:, 1:2], in1=scl_sb[:, j : j + 1],
                        op=AL.mult,
                    )
                    nc.vector.tensor_tensor(
                        out=bj, in0=ppc[:, 0:1], in1=aj, op=AL.mult
                    )
                    nc.vector.tensor_tensor(
                        out=bj, in0=bia_sb[:, j : j + 1], in1=bj, op=AL.subtract
                    )
                # apply: 2 tiles on ACT (idle pre-loop), 2 on DVE
                for j in range(CT):
                    if j < 2:
                        nc.scalar.activation(
                            out=xt8[:, j * T : (j + 1) * T],
                            in_=xt[:, j * T : (j + 1) * T],
                            func=AF.Identity,
                            scale=ab[:, 2 * j : 2 * j + 1],
                            bias=ab[:, 2 * j + 1 : 2 * j + 2],
                        )
                    else:
                        nc.vector.tensor_scalar(
                            out=xt8[:, j * T : (j + 1) * T],
                            in0=xt[:, j * T : (j + 1) * T],
                            scalar1=ab[:, 2 * j : 2 * j + 1],
                            scalar2=ab[:, 2 * j + 1 : 2 * j + 2],
                            op0=AL.mult,
                            op1=AL.add,
                        )

            # =========== Stage 2: v / q / k projections (fp8 DR) ==========
            # Emission order tuned for the engine FIFOs: pair-0 q/k first
            # (bias on ACT so the attention loop can start early), then vT
            # (bias on DVE), then pairs 1-3 q/k (bias on DVE, overlapping
            # the start of the attention loop).
            with (
                tc.tile_pool(name="s2", bufs=1) as s2,
                tc.tile_pool(name="s2p", bufs=1, space="PSUM") as s2p,
            ):
                inv_sw = s2.tile([128, 1], f32, name="inv_sw")
                nc.vector.memset(inv_sw[:, :], 1.0 / SW)

                w4q = wq_sb.rearrange("p (j c) -> p j c", j=CT)
                w4k = wk_sb.rearrange("p (j c) -> p j c", j=CT)
                x4 = xt8.rearrange("p (j t) -> p j t", j=CT)
                wv4 = wv_sb.rearrange("p (j c) -> p j c", j=CT)

                def qk_mms(w4, ot):
                    ps = s2p.tile([128, T], f32, name="pq", tag="pq", bufs=3)
                    for kp in range(2):
                        for nt in range(2):
                            nc.tensor.matmul(
                                ps[:, nt * 512 : (nt + 1) * 512],
                                lhsT=w4[:, 2 * kp : 2 * kp + 2,
                                        ot * 128 : (ot + 1) * 128],
                                rhs=x4[:, 2 * kp : 2 * kp + 2,
                                       nt * 512 : (nt + 1) * 512],
                                start=(kp == 0), stop=(kp == 1), perf_mode=DR,
                            )
                    return ps

                def qk_bias_act(ps, b_sb, dst, ot):
                    nc.scalar.activation(
                        out=dst[:, ot * T : (ot + 1) * T],
                        in_=ps[:, :],
                        func=AF.Identity,
                        scale=1.0 / SW,
                        bias=b_sb[:, ot : ot + 1],
                    )

                def qk_bias_dve(ps, b_sb, dst, ot):
                    nc.vector.tensor_scalar(
                        out=dst[:, ot * T : (ot + 1) * T],
                        in0=ps[:, :],
                        scalar1=inv_sw[:, 0:1],
                        scalar2=b_sb[:, ot : ot + 1],
                        op0=AL.mult,
                        op1=AL.add,
                    )

                def vt_emit(tb):
                    # no v bias here: softmax weights sum to 1, so the v
                    # bias rides through the attention average and its
                    # projection image is folded into pb host-side
                    pv2 = s2p.tile([128, 512], f32, name="pv2", tag="pv",
                                   bufs=2)
                    for kp in range(2):
                        nc.tensor.matmul(
                            pv2[:, :],
                            lhsT=x4[:, 2 * kp : 2 * kp + 2,
                                    tb * 128 : (tb + 1) * 128],
                            rhs=wv4[:, 2 * kp : 2 * kp + 2, :],
                            start=(kp == 0), stop=(kp == 1), perf_mode=DR,
                        )
                    # all on ACT (idle pre-loop): keeps the vT matmul
                    # stream off the DVE bias queue entirely
                    nc.scalar.activation(
                        out=vta8[:, tb * C : (tb + 1) * C],
                        in_=pv2[:, :],
                        func=AF.Copy,
                        scale=1.0 / SW,
                    )

                # pair-0 q/k first: bias on ACT, so the attention loop's
                # first scores are only ~2us behind xt8
                ps = qk_mms(w4q, 0)
                qk_bias_act(ps, bq_sb, q16, 0)
                ps = qk_mms(w4k, 0)
                qk_bias_act(ps, bk_sb, k16, 0)
                # pairs 1-3 q/k next: pair-0 scores wait only on the pq
                # rotation, which these biases release quickly
                for ot in range(1, CT):
                    ps = qk_mms(w4q, ot)
                    qk_bias_dve(ps, bq_sb, q16, ot)
                    ps = qk_mms(w4k, ot)
                    qk_bias_dve(ps, bk_sb, k16, ot)
                # vT last: AV consumes vta tb k only at sb k of pair 0,
                # well after the loop starts
                for tb in range(TB):
                    vt_emit(tb)

            # =========== Stage 3: attention ===========
            # PSUM: sc tag bufs=2 (4 banks) + pav (2) + pd (2) = 8 banks.
            # Flat software-pipelined emission: each slot emits its score
            # matmuls FIRST, then pops one deferred AV/d/tail closure from
            # the previous slot — so the PE FIFO never blocks the next
            # exp's input behind AV work that waits on the current exps.
            with (
                tc.tile_pool(name="s3", bufs=1) as s3,
                tc.tile_pool(name="s3p", bufs=1, space="PSUM") as s3p,
            ):
                from collections import deque

                et4 = vta8.rearrange("p (t c) -> p t c", t=TB)
                o2 = ones8_sb.rearrange("p (two o) -> p two o", two=2)
                pending = deque()
                pcur = {}

                def ensure_pav(sbp_or_sb0):
                    if sbp_or_sb0:
                        return
                    pcur["pav"] = s3p.tile(
                        [128, T], f32, name="pav", tag="pav", bufs=1
                    )
                    pcur["pd"] = s3p.tile(
                        [128, T], f32, name="pd", tag="pd", bufs=1
                    )

                def make_b(pr, sb, et8):
                    # B side: non-DR (DR cannot write dst partition base
                    # 64), one sb per matmul — small closure (~4 matmuls)
                    def emit():
                        ensure_pav(sb)
                        pav, pdt = pcur["pav"], pcur["pd"]
                        half = (sb % 2) * 2048
                        lB1 = et4[
                            :, sb, (2 * pr + 1) * CH : (2 * pr + 2) * CH
                        ]
                        stB = dict(start=(sb == 0), stop=(sb == TB - 1))
                        # same-lhsT matmuls adjacent: walrus elides the
                        # repeated Ldweights
                        for nt in range(2):
                            ncol = slice(nt * 512, (nt + 1) * 512)
                            rB = et8[:, half + 1024 + nt * 512 :
                                     half + 1024 + (nt + 1) * 512]
                            nc.tensor.matmul(
                                pav[CH:128, ncol], lhsT=lB1, rhs=rB, **stB
                            )
                        for nt in range(2):
                            ncol = slice(nt * 512, (nt + 1) * 512)
                            rB = et8[:, half + 1024 + nt * 512 :
                                     half + 1024 + (nt + 1) * 512]
                            nc.tensor.matmul(
                                pdt[CH:128, ncol],
                                lhsT=ones8_sb[:, 0:CH],
                                rhs=rB, **stB,
                            )

                    return emit

                def make_a(pr, sbp, et8):
                    def emit():
                        pav, pdt = pcur["pav"], pcur["pd"]
                        e2 = et8.rearrange("p (two c) -> p two c", two=2)
                        lA = et4[
                            :, 2 * sbp : 2 * sbp + 2,
                            2 * pr * CH : (2 * pr + 1) * CH,
                        ]
                        stA = dict(
                            start=(sbp == 0), stop=(sbp == NP - 1),
                            perf_mode=DR,
                        )
                        for nt in range(2):
                            ncol = slice(nt * 512, (nt + 1) * 512)
                            nc.tensor.matmul(
                                pav[0:CH, ncol],
                                lhsT=lA,
                                rhs=e2[:, :, nt * 512 : (nt + 1) * 512],
                                **stA,
                            )
                        # denominator, pre-broadcast over 64 partitions
                        # (ones value 1/SA folds the ar scale in)
                        for nt in range(2):
                            ncol = slice(nt * 512, (nt + 1) * 512)
                            nc.tensor.matmul(
                                pdt[0:CH, ncol],
                                lhsT=o2,
                                rhs=e2[:, :, nt * 512 : (nt + 1) * 512],
                                **stA,
                            )

                    return emit

                def make_tail(pr):
                    def emit():
                        pav, pdt = pcur["pav"], pcur["pd"]
                        rdrep = s3.tile(
                            [128, T], f32, name="rdrep", tag="rd", bufs=2
                        )
                        y0 = s3.tile([128, T], f32, name="y0", tag="y0",
                                     bufs=2)
                        # ~0.4%-accurate reciprocal from standard DVE ops:
                        # BITWISE_NOT seed + Chebyshev-tuned NR pass
                        # (constants from dve_ops.RECIP_APPROX_FAST_CONSTS);
                        # nc.vector.reciprocal is ~6 cycles/elem, too slow.
                        # The last pair runs in column halves so stage 4's
                        # nt0 chain starts after only half the chain.
                        i32 = mybir.dt.int32
                        halves = (
                            (slice(0, 512), slice(512, T))
                            if pr == NP - 1
                            else (slice(0, T),)
                        )
                        for hs in halves:
                            nc.vector.tensor_scalar(
                                out=y0[:, hs].bitcast(i32),
                                in0=pdt[:, hs].bitcast(i32),
                                scalar1=-1, scalar2=None, op0=AL.bitwise_xor,
                            )
                            nc.vector.tensor_scalar(
                                out=y0[:, hs], in0=y0[:, hs],
                                scalar1=-0.23549792, scalar2=None,
                                op0=AL.mult,
                            )
                            nc.vector.tensor_tensor(
                                out=rdrep[:, hs], in0=pdt[:, hs],
                                in1=y0[:, hs], op=AL.mult,
                            )
                            nc.vector.tensor_scalar(
                                out=rdrep[:, hs], in0=rdrep[:, hs],
                                scalar1=-1.0, scalar2=2.0017324,
                                op0=AL.mult, op1=AL.add,
                            )
                            nc.vector.tensor_tensor(
                                out=rdrep[:, hs], in0=rdrep[:, hs],
                                in1=y0[:, hs], op=AL.mult,
                            )
                            nc.vector.tensor_tensor(
                                out=ar8[:, pr * T : (pr + 1) * T][:, hs],
                                in0=pav[:, hs],
                                in1=rdrep[:, hs],
                                op=AL.mult,
                            )
                        if debug:
                            pdc = s3.tile(
                                [128, T], f32, name="pdc", tag="pdc", bufs=2
                            )
                            nc.vector.tensor_copy(out=pdc[:, :], in_=pdt[:, :])
                            nc.sync.dma_start(
                                out=dbg_pd[pr * 128 : (pr + 1) * 128, :],
                                in_=pdc[:, :],
                            )

                    return emit

                two_c = s3.tile([128, 1], f32, name="two_c")
                nc.vector.memset(two_c[:, :], 2.0)

                def dve_exp(sc, et_out):
                    # exp via degree-3 polynomial on s/4 then two squarings
                    # (all standard DVE ops) — offloads the ACT wall.
                    # p3(x) = (1/6)((x^2+2x+4)(x+1)+2) = 1+x+x^2/2+x^3/6;
                    # exp(s) ~ p3(s/4)^4, rel err <~1% for |s|<2.
                    px = s3.tile([128, T], f32, name="px", tag="px", bufs=2)
                    py = s3.tile([128, T], f32, name="py", tag="py", bufs=2)
                    nc.vector.tensor_scalar(
                        out=px[:, :], in0=sc[:, :],
                        scalar1=0.25, scalar2=None, op0=AL.mult,
                    )
                    nc.vector.tensor_scalar(
                        out=py[:, :], in0=px[:, :],
                        scalar1=1.0, scalar2=None, op0=AL.add,
                    )
                    nc.vector.scalar_tensor_tensor(
                        out=px[:, :], in0=px[:, :], scalar=two_c[:, 0:1],
                        in1=px[:, :], op0=AL.add, op1=AL.mult,
                    )
                    nc.vector.tensor_scalar(
                        out=px[:, :], in0=px[:, :],
                        scalar1=4.0, scalar2=None, op0=AL.add,
                    )
                    nc.vector.tensor_tensor(
                        out=py[:, :], in0=px[:, :], in1=py[:, :], op=AL.mult
                    )
                    nc.vector.tensor_scalar(
                        out=py[:, :], in0=py[:, :],
                        scalar1=1.0 / 6.0, scalar2=1.0 / 3.0,
                        op0=AL.mult, op1=AL.add,
                    )
                    nc.vector.tensor_tensor(
                        out=py[:, :], in0=py[:, :], in1=py[:, :], op=AL.mult
                    )
                    nc.vector.tensor_tensor(
                        out=et_out, in0=py[:, :], in1=py[:, :], op=AL.mult
                    )

                et8 = None
                b_del = None
                delayed = []
                OFF_SB, OFF_NT = 4, 1  # one DVE-offloaded exp tile per pair
                for gsb in range(NH * TB // 2):
                    pr, sb = divmod(gsb, TB)
                    for item in [d for d in delayed if d[0] <= gsb]:
                        pending.append(item[1])
                        delayed.remove(item)
                    if b_del is not None:
                        pending.append(b_del)
                        b_del = None
                    qA = q16[0:CH, pr * T : (pr + 1) * T]
                    kA = k16[0:CH, pr * T : (pr + 1) * T]
                    qB = q16[CH:128, pr * T : (pr + 1) * T]
                    kB = k16[CH:128, pr * T : (pr + 1) * T]
                    if sb % 2 == 0:
                        et8 = s3.tile(
                            [128, 4096], fp8, name="et8", tag="et", bufs=2
                        )
                    for nt in range(2):
                        sc = s3p.tile([128, T], f32, name="sc", tag="sc",
                                      bufs=2)
                        nc.tensor.matmul(
                            sc[:, 0:512],
                            lhsT=kA[:, sb * 128 : (sb + 1) * 128],
                            rhs=qA[:, nt * 512 : (nt + 1) * 512],
                            start=True, stop=True,
                        )
                        nc.tensor.matmul(
                            sc[:, 512:1024],
                            lhsT=kB[:, sb * 128 : (sb + 1) * 128],
                            rhs=qB[:, nt * 512 : (nt + 1) * 512],
                            start=True, stop=True,
                        )
                        # one small deferred closure per nt slot keeps the
                        # PE bursts short so ACT never starves
                        if pending:
                            pending.popleft()()
                        # out: A-ntX -> et8[half + nt*512],
                        #      B-ntX -> et8[half + 1024 + nt*512]
                        et_out = et8[:, :].rearrange(
                            "p (h ab n) -> p h ab n", h=2, ab=2
                        )[:, sb % 2, :, nt * 512 : (nt + 1) * 512]
                        if sb == OFF_SB and nt == OFF_NT:
                            dve_exp(sc, et_out)
                        else:
                            nc.scalar.activation(
                                out=et_out, in_=sc[:, :], func=AF.Exp
                            )
                    if sb == OFF_SB:
                        # consumers of the DVE-produced tile wait one extra
                        # slot so the PE FIFO never blocks on the chain
                        delayed.append((gsb + 2, make_b(pr, sb, et8)))
                    else:
                        b_del = make_b(pr, sb, et8)
                    if sb % 2 == 1:
                        mk_a = make_a(pr, sb // 2, et8)
                        if sb // 2 == OFF_SB // 2:
                            delayed.append((gsb + 2, mk_a))
                        elif sb == TB - 1:
                            # the tail's reciprocal reads pd: the last B
                            # contribution must be emitted before it
                            pending.append(b_del)
                            b_del = None
                            pending.append(mk_a)
                            pending.append(make_tail(pr))
                        else:
                            pending.append(mk_a)
                        if sb == TB - 1 and sb // 2 == OFF_SB // 2:
                            pending.append(b_del)
                            b_del = None
                            pending.append(make_tail(pr))
                if b_del is not None:
                    pending.append(b_del)
                for _, cl in delayed:
                    pending.append(cl)
                while pending:
                    pending.popleft()()

            # =========== Stage 4: proj + residual ===========
            with (
                tc.tile_pool(name="s4", bufs=1) as s4,
                tc.tile_pool(name="s4p", bufs=1, space="PSUM") as s4p,
            ):
                inv_o = s4.tile([128, 1], f32, name="inv_o")
                nc.vector.memset(inv_o[:, :], 1.0 / (SW * SA))
                # xpb = xt + proj bias: emitted here so the DVE ops run in
                # loop slack, well before the stts below need them
                for j in range(CT):
                    nc.vector.tensor_scalar(
                        out=xpb[:, j * T : (j + 1) * T],
                        in0=xt[:, j * T : (j + 1) * T],
                        scalar1=pb_sb[:, j : j + 1],
                        scalar2=None,
                        op0=AL.add,
                    )
                pw4 = pw_sb.rearrange("p (j c) -> p j c", j=CT)
                ar4 = ar8.rearrange("p (j t) -> p j t", j=CT)
                for nt in range(2):
                    # nt-major: the nt0 chain starts as soon as the last
                    # pair's first normalization half lands in ar8
                    for j in range(CT):
                        po = s4p.tile(
                            [128, 512], f32, name="po", tag="po", bufs=4
                        )
                        for kp in range(2):
                            nc.tensor.matmul(
                                po[:, :],
                                lhsT=pw4[:, 2 * kp : 2 * kp + 2,
                                         j * 128 : (j + 1) * 128],
                                rhs=ar4[:, 2 * kp : 2 * kp + 2,
                                        nt * 512 : (nt + 1) * 512],
                                start=(kp == 0), stop=(kp == 1), perf_mode=DR,
                            )
                        ot_ = s4.tile([128, 512], f32, name="ot_", tag="ot",
                                      bufs=4)
                        xpb_sl = xpb[:, j * T + nt * 512 :
                                     j * T + nt * 512 + 512]
                        if j % 2 == 0:
                            nc.vector.scalar_tensor_tensor(
                                out=ot_[:, :],
                                in0=po[:, :],
                                scalar=inv_o[:, 0:1],
                                in1=xpb_sl,
                                op0=AL.mult,
                                op1=AL.add,
                            )
                        else:
                            # tail is DVE-serial otherwise: route half the
                            # chunks through idle ACT (scale) + GPSIMD (add)
                            pc = s4.tile([128, 512], f32, name="pc", tag="pc",
                                         bufs=2)
                            nc.scalar.activation(
                                out=pc[:, :], in_=po[:, :], func=AF.Copy,
                                scale=1.0 / (SW * SA),
                            )
                            nc.gpsimd.tensor_tensor(
                                out=ot_[:, :], in0=pc[:, :], in1=xpb_sl,
                                op=AL.add,
                            )
                        # loop mode: y owns the scalar queue (drains into
                        # the next iteration); single-shot spreads over 3
                        if loop_n:
                            eng = dma2
                        else:
                            eng = (nc.sync, dma2, nc.gpsimd, nc.sync)[j]
                        eng.dma_start(
                            out=y[j * 128 : (j + 1) * 128,
                                  nt * 512 : (nt + 1) * 512],
                            in_=ot_[:, :],
                        )

            if debug:
                for j in range(CT):
                    sl = slice(j * 128, (j + 1) * 128)
                    ts = slice(j * T, (j + 1) * T)
                    nc.sync.dma_start(
                        out=dbg_xt8[sl, :], in_=xt8[:, ts].bitcast(u8)
                    )
                    nc.sync.dma_start(out=dbg_q[sl, :], in_=q16[:, ts])
                    nc.sync.dma_start(out=dbg_k[sl, :], in_=k16[:, ts])
                    nc.sync.dma_start(
                        out=dbg_ar[sl, :], in_=ar8[:, ts].bitcast(u8)
                    )
                nc.sync.dma_start(out=dbg_v[:, :], in_=vta8[:, :].bitcast(u8))

            loop_cm.__exit__(None, None, None)

    return nc


def _prep_host(norm_scale, norm_bias, qkv_w, qkv_b, proj_w, proj_b):
    import ml_dtypes

    f8 = ml_dtypes.float8_e4m3
    s = float(CH) ** -0.25
    w3 = qkv_w.reshape(NH, 3, CH, C)
    b3 = qkv_b.reshape(NH, 3, CH)

    def to8(a):
        return np.ascontiguousarray(a).astype(f8).view(np.uint8)

    wq = to8((w3[:, 0] * s).reshape(C, C).T * SW)
    wk = to8((w3[:, 1] * s).reshape(C, C).T * SW)
    wv = to8(w3[:, 2].reshape(C, C).T * SW)
    pw = to8(proj_w.T * SW)
    # all biases are added AFTER the 1/SW rescale, so they stay unscaled.
    # v bias: softmax weights sum to 1, so its contribution to the
    # attention output is the constant vector bv, folded into pb here.
    bq_ = np.ascontiguousarray((b3[:, 0] * s).reshape(C))
    bk_ = np.ascontiguousarray((b3[:, 1] * s).reshape(C))
    bv_ = np.ascontiguousarray(b3[:, 2].reshape(C))
    pb_ = np.ascontiguousarray(proj_b + proj_w @ bv_)
    c = np.arange(C)
    gmap_ = (c[:, None] // GS == np.arange(NG)[None, :]).astype(np.float32)
    emap_ = np.ascontiguousarray(gmap_.T)
    ones8_ = np.full((128, 128), 1.0 / SA, dtype=np.float32).astype(f8).view(
        np.uint8
    )
    return {
        "wq8": wq, "wk8": wk, "wv8": wv, "pw8": pw,
        "bq": bq_.astype(np.float32), "bk": bk_.astype(np.float32),
        "pb": pb_.astype(np.float32),
        "nsc": norm_scale.astype(np.float32),
        "nbi": norm_bias.astype(np.float32),
        "gmap": gmap_, "emap": emap_, "ones8": ones8_,
    }


def make_in_maps(x, norm_scale, norm_bias, qkv_w, qkv_b, proj_w, proj_b):
    shared = _prep_host(norm_scale, norm_bias, qkv_w, qkv_b, proj_w, proj_b)
    in_maps = []
    for b in range(N_CORES):
        m = dict(shared)
        m["x"] = np.ascontiguousarray(x[b].reshape(C, T).astype(np.float32))
        in_maps.append(m)
    return in_maps


def get_nc(split_waits=True, loop_n=None):
    key = ("nc2", split_waits, loop_n)
    if key not in _CACHE:
        from concourse import mybir
        import bass_rust

        nc = build_nc(loop_n=loop_n)
        if split_waits:
            _split_excess_waits(nc, mybir, bass_rust)
        _CACHE[key] = nc
    return _CACHE[key]


def kernel(x, norm_scale, norm_bias, qkv_w, qkv_b, proj_w, proj_b):
    from concourse.bass_utils import run_bass_kernel_spmd

    # NOTE: no LDW-opt patch here — the walrus LDW optimization rejects
    # bf16/fp8 Ldweights, and this kernel's matmuls are all bf16/fp8.
    nc = get_nc()
    in_maps = make_in_maps(
        x, norm_scale, norm_bias, qkv_w, qkv_b, proj_w, proj_b
    )
    res = run_bass_kernel_spmd(nc, in_maps, core_ids=list(range(N_CORES)))
    out = np.stack([res.results[b]["y"] for b in range(N_CORES)], axis=0)
    return out.reshape(B, C, 32, 32).astype(np.float32)


# revision 6
# speedup vs baseline: 1.0437x; 1.0437x over previous
"""AttentionBlock kernel v2 for 8x Trainium2 NeuronCores.

Data-parallel over batch: core b computes batch element b end-to-end.
Per core: x [512, 1024] -> GroupNorm(32) -> qkv -> 8-head attention -> proj
+ residual -> y [512, 1024].

v2 structure (from HW microbenchmarks + cost model):
  - fp8e4m3 weights (scaled x32, folded back in the bias step) and fp8
    activations for v/proj; q/k kept bf16. Halves weight DMA 4x and
    enables DoubleRow (K=256) matmuls at 0.5 cyc/row for qkv/AV/proj.
  - exp is the hard floor (ACT fixed-rate ~1.1us per [128,1024] tile,
    64 tiles): the loop is ACT-bound, PE has slack, so all projections
    run outside the loop, no filler machinery.
  - PSUM: score tag bufs=2 (4 banks) + one [128,1024] AV accumulator
    (A-head rows 0:64, B-head rows 64:128 via DR out-partition offset)
    + aux tag (pd denominators / prn broadcast) = 8 banks exactly.
  - softmax denominator rides tiny ones-lhsT DR matmuls into pd;
    per-pair normalization (recip + PE broadcast + one DVE mult into
    fp8 ar) overlaps the next pair's ACT-bound phase.
  - residual uses the resident fp32 x (no DRAM reload); xpb = x + proj
    bias precomputed on DVE during the loop.
"""

import sys

sys.path.insert(0, "/opt/trn_rl_repo")

import numpy as np

B, C, T = 8, 512, 1024
NH, CH = 8, 64
NG, GS = 32, 16
EPS = 1e-5
N_CORES = 8
CT = C // 128  # channel tiles (4)
TB = T // 128  # s blocks (8)
NP = NH // 2  # head pairs (4)
SW = 32.0  # weight scale (fp8 subnormal avoidance)
SA = 4.0  # ar scale

_CACHE = {}


def _install_compile_patches():
    from concourse import bass_utils

    if getattr(bass_utils, "_ldw_opt_patched", False):
        return
    orig = bass_utils.run_command

    def patched(cmd, **kw):
        cmd = [
            c.replace("--enable-ldw-opt=false", "--enable-ldw-opt=true")
            if isinstance(c, str)
            else c
            for c in cmd
        ]
        return orig(cmd, **kw)

    bass_utils.run_command = patched
    bass_utils._ldw_opt_patched = True


def _install_tile_drain_patch(tile_mod, vector_clock_mod, bass_rust_mod):
    """Split TileContext's exit-drain waits over multiple SP nops (CTRL
    instructions accept a single sync wait on this walrus)."""
    ScopedClock = vector_clock_mod.ScopedClock

    def _patched(self, tick_clock, wait_clock):
        nc = self.nc
        probe = nc.sync.nop(nofuse=True)
        wait_clock.add_sem_waits(
            probe.ins, ScopedClock({None: tick_clock.global_clock})
        )
        waits = list(probe.ins.sync_info.on_wait) if probe.ins.sync_info else []
        probe.ins.sync_info = bass_rust_mod.SyncInfo(
            on_wait=waits[:1], on_update=[]
        )
        for w in waits[1:]:
            extra = nc.sync.nop(nofuse=True)
            extra.ins.sync_info = bass_rust_mod.SyncInfo(
                on_wait=[w], on_update=[]
            )
        nc.sync.drain()
        nc.all_engine_barrier()
        assert self.sems is not None
        popped = nc._tile_sem_poison_stack.pop()
        assert popped is self._sem_poison
        nc.clear_and_free_semaphores(list(self.sems.allocated().values()))
        nc.all_engine_barrier()

    tile_mod.TileContext._drain_and_barrier = _patched


def _split_excess_waits(nc, mybir, bass_rust, cap=1):
    cnt = 0
    for fn in nc.m.functions:
        for bb in fn.blocks:
            il = bb.instructions
            new_list = []
            for ins in il:
                si = ins.sync_info
                waits = list(si.on_wait) if si and si.on_wait else []
                if len(waits) > cap:
                    for w in waits[:-cap]:
                        cnt += 1
                        new_list.append(
                            mybir.InstNoOp(
                                name=f"waitsplit-{cnt}",
                                engine=ins.engine,
                                ins=[],
                                outs=[],
                                sync_info=bass_rust.SyncInfo(
                                    on_wait=[w], on_update=[]
                                ),
                            )
                        )
                    ins.sync_info = bass_rust.SyncInfo(
                        on_wait=waits[-cap:],
                        on_update=list(si.on_update) if si.on_update else [],
                    )
                new_list.append(ins)
            il[:] = new_list
    return cnt


def build_nc(loop_n=None, debug=False):
    from contextlib import nullcontext
    from concourse import bass, mybir, tile
    from concourse import vector_clock
    import bass_rust

    _install_tile_drain_patch(tile, vector_clock, bass_rust)

    f32 = mybir.dt.float32
    f32r = mybir.dt.float32r
    bf16 = mybir.dt.bfloat16
    fp8 = mybir.dt.float8e4
    u8 = mybir.dt.uint8
    AL = mybir.AluOpType
    AF = mybir.ActivationFunctionType
    DR = mybir.MatmulPerfMode.DoubleRow

    nc = bass.Bass(num_devices=N_CORES)

    # --- I/O (fp8 weights travel as uint8, bitcast on DMA) ---
    x = nc.declare_dram_parameter("x", [C, T], f32, isOutput=False)
    wq8 = nc.declare_dram_parameter("wq8", [C, C], u8, isOutput=False)  # [c,o]
    wk8 = nc.declare_dram_parameter("wk8", [C, C], u8, isOutput=False)
    wv8 = nc.declare_dram_parameter("wv8", [C, C], u8, isOutput=False)
    pw8 = nc.declare_dram_parameter("pw8", [C, C], u8, isOutput=False)
    bq = nc.declare_dram_parameter("bq", [C], f32, isOutput=False)  # pre-scaled
    bk = nc.declare_dram_parameter("bk", [C], f32, isOutput=False)
    pb = nc.declare_dram_parameter("pb", [C], f32, isOutput=False)
    nsc = nc.declare_dram_parameter("nsc", [C], f32, isOutput=False)
    nbi = nc.declare_dram_parameter("nbi", [C], f32, isOutput=False)
    gmap = nc.declare_dram_parameter("gmap", [C, NG], f32, isOutput=False)
    emap = nc.declare_dram_parameter("emap", [NG, C], f32, isOutput=False)
    ones8 = nc.declare_dram_parameter("ones8", [128, 128], u8, isOutput=False)
    y = nc.declare_dram_parameter("y", [C, T], f32, isOutput=True)
    if debug:
        dbg_xt8 = nc.declare_dram_parameter("dbg_xt8", [C, T], u8, isOutput=True)
        dbg_q = nc.declare_dram_parameter("dbg_q", [C, T], bf16, isOutput=True)
        dbg_k = nc.declare_dram_parameter("dbg_k", [C, T], bf16, isOutput=True)
        dbg_v = nc.declare_dram_parameter("dbg_v", [128, TB * C], u8, isOutput=True)
        dbg_ar = nc.declare_dram_parameter("dbg_ar", [C, T], u8, isOutput=True)
        dbg_pd = nc.declare_dram_parameter("dbg_pd", [NP * 128, T], f32, isOutput=True)

    with tile.TileContext(nc) as tc:
        with tc.tile_pool(name="persist", bufs=1) as pp:
            # --- persistent SBUF ---
            xt = pp.tile([128, CT * T], f32, name="xt")
            xpb = pp.tile([128, CT * T], f32, name="xpb")
            xt8 = pp.tile([128, CT * T], fp8, name="xt8")
            wq_sb = pp.tile([128, CT * C], fp8, name="wq_sb")
            wk_sb = pp.tile([128, CT * C], fp8, name="wk_sb")
            wv_sb = pp.tile([128, CT * C], fp8, name="wv_sb")
            pw_sb = pp.tile([128, CT * C], fp8, name="pw_sb")
            q16 = pp.tile([128, CT * T], bf16, name="q16")
            k16 = pp.tile([128, CT * T], bf16, name="k16")
            vta8 = pp.tile([128, TB * C], fp8, name="vta8")
            ar8 = pp.tile([128, CT * T], fp8, name="ar8")
            bq_sb = pp.tile([128, CT], f32, name="bq_sb")
            bk_sb = pp.tile([128, CT], f32, name="bk_sb")
            pb_sb = pp.tile([128, CT], f32, name="pb_sb")
            scl_sb = pp.tile([128, CT], f32, name="scl_sb")
            bia_sb = pp.tile([128, CT], f32, name="bia_sb")
            gmap_sb = pp.tile([128, CT * NG], f32, name="gmap_sb")
            emap_sb = pp.tile([NG, C], f32, name="emap_sb")
            ones8_sb = pp.tile([128, 128], fp8, name="ones8_sb")

            loop_cm = tc.For_i(0, loop_n, 1) if loop_n else nullcontext()

            # --- loop-invariant loads.  Queue plan (x must land ASAP, its
            # queue order is behind the loop-invariant weights):
            #   sync:   wq (1 desc), then per-iteration x halves
            #   scalar: wk (1 desc), then x halves
            #   SWDGE:  wv, consts, pw (wv needed ~12us, pw only at stage 4)
            nc.sync.dma_start(
                out=wq_sb[:, :].rearrange("p (j c) -> p j c", j=CT).bitcast(u8),
                in_=wq8[:, :].rearrange("(j p) c -> p j c", j=CT),
            )
            nc.scalar.dma_start(
                out=wk_sb[:, :].rearrange("p (j c) -> p j c", j=CT).bitcast(u8),
                in_=wk8[:, :].rearrange("(j p) c -> p j c", j=CT),
            )
            nc.gpsimd.dma_start(
                out=wv_sb[:, :].rearrange("p (j c) -> p j c", j=CT).bitcast(u8),
                in_=wv8[:, :].rearrange("(j p) c -> p j c", j=CT),
            )
            for dst, srcp in (
                (bq_sb, bq), (bk_sb, bk), (pb_sb, pb),
                (scl_sb, nsc), (bia_sb, nbi),
            ):
                nc.gpsimd.dma_start(
                    out=dst[:, :], in_=srcp[:].rearrange("(j p) -> p j", j=CT)
                )
            nc.gpsimd.dma_start(
                out=gmap_sb[:, :].rearrange("p (j g) -> p j g", j=CT),
                in_=gmap[:, :].rearrange("(j p) g -> p j g", j=CT),
            )
            nc.gpsimd.dma_start(out=emap_sb[:, :], in_=emap[:, :])
            nc.gpsimd.dma_start(
                out=ones8_sb[:, :].bitcast(u8), in_=ones8[:, :]
            )
            nc.gpsimd.dma_start(
                out=pw_sb[:, :].rearrange("p (j c) -> p j c", j=CT).bitcast(u8),
                in_=pw8[:, :].rearrange("(j p) c -> p j c", j=CT),
            )

            loop_cm.__enter__()
            dma2 = nc.scalar
            for j in range(CT):
                # loop mode: x owns the sync queue exclusively so the next
                # iteration's x-load prefetches during this iteration's
                # attention phase instead of queueing behind y-out
                eng = nc.sync if (loop_n or j % 2 == 0) else dma2
                eng.dma_start(
                    out=xt[:, j * T : (j + 1) * T],
                    in_=x[j * 128 : (j + 1) * 128, :],
                )

            # =========== Stage 1: GroupNorm -> xt8 (fp8) ===========
            with (
                tc.tile_pool(name="s1", bufs=1) as s1,
                tc.tile_pool(name="s1p", bufs=1, space="PSUM") as s1p,
            ):
                stats2 = s1.tile([128, 2 * CT], f32, name="stats2")
                for j in range(CT):
                    xtj = xt[:, j * T : (j + 1) * T]
                    nc.vector.tensor_reduce(
                        out=stats2[:, 2 * j : 2 * j + 1],
                        in_=xtj,
                        axis=mybir.AxisListType.X,
                        op=AL.add,
                    )
                    scr = s1.tile([128, T], f32, name="sq_scr", tag="sq", bufs=2)
                    nc.scalar.activation(
                        out=scr,
                        in_=xtj,
                        func=AF.Square,
                        accum_out=stats2[:, 2 * j + 1 : 2 * j + 2],
                    )
                pst = s1p.tile([NG, 2], f32, name="pst")
                for j in range(CT):
                    nc.tensor.matmul(
                        pst[:, :],
                        lhsT=gmap_sb[:, j * NG : (j + 1) * NG],
                        rhs=stats2[:, 2 * j : 2 * j + 2],
                        start=(j == 0),
                        stop=(j == CT - 1),
                    )
                grp = s1.tile([NG, 8], f32, name="grp")
                inv_n = 1.0 / (GS * T)
                nc.vector.tensor_scalar(
                    out=grp[:, 0:1], in0=pst[:, 0:1],
                    scalar1=inv_n, scalar2=None, op0=AL.mult,
                )
                nc.vector.tensor_scalar(
                    out=grp[:, 2:3], in0=pst[:, 1:2],
                    scalar1=inv_n, scalar2=None, op0=AL.mult,
                )
                nc.vector.tensor_tensor(
                    out=grp[:, 3:4], in0=grp[:, 0:1], in1=grp[:, 0:1], op=AL.mult
                )
                nc.vector.tensor_tensor(
                    out=grp[:, 2:3], in0=grp[:, 2:3], in1=grp[:, 3:4],
                    op=AL.subtract,
                )
                nc.vector.tensor_scalar(
                    out=grp[:, 2:3], in0=grp[:, 2:3],
                    scalar1=EPS, scalar2=None, op0=AL.add,
                )
                nc.scalar.activation(
                    out=grp[:, 3:4], in_=grp[:, 2:3], func=AF.Sqrt, bias=0.0
                )
                nc.vector.reciprocal(out=grp[:, 1:2], in_=grp[:, 3:4])

                ab = s1.tile([128, 2 * CT], f32, name="ab")
                for j in range(CT):
                    ppc = s1p.tile([128, 2], f32, name="ppc", tag="ppc", bufs=2)
                    nc.tensor.matmul(
                        ppc[:, :],
                        lhsT=emap_sb[:, j * 128 : (j + 1) * 128],
                        rhs=grp[:, 0:2],
                        start=True,
                        stop=True,
                    )
                    aj = ab[:, 2 * j : 2 * j + 1]
                    bj = ab[:, 2 * j + 1 : 2 * j + 2]
                    nc.vector.tensor_tensor(
                        out=aj, in0=ppc[:, 1:2], in1=scl_sb[:, j : j + 1],
                        op=AL.mult,
                    )
                    nc.vector.tensor_tensor(
                        out=bj, in0=ppc[:, 0:1], in1=aj, op=AL.mult
                    )
                    nc.vector.tensor_tensor(
                        out=bj, in0=bia_sb[:, j : j + 1], in1=bj, op=AL.subtract
                    )
                # apply: 2 tiles on ACT (idle pre-loop), 2 on DVE
                for j in range(CT):
                    if j < 2:
                        nc.scalar.activation(
                            out=xt8[:, j * T : (j + 1) * T],
                            in_=xt[:, j * T : (j + 1) * T],
                            func=AF.Identity,
                            scale=ab[:, 2 * j : 2 * j + 1],
                            bias=ab[:, 2 * j + 1 : 2 * j + 2],
                        )
                    else:
                        nc.vector.tensor_scalar(
                            out=xt8[:, j * T : (j + 1) * T],
                            in0=xt[:, j * T : (j + 1) * T],
                            scalar1=ab[:, 2 * j : 2 * j + 1],
                            scalar2=ab[:, 2 * j + 1 : 2 * j + 2],
                            op0=AL.mult,
                            op1=AL.add,
                        )

            # =========== Stage 2: v / q / k projections (fp8 DR) ==========
            # Emission order tuned for the engine FIFOs: pair-0 q/k first
            # (bias on ACT so the attention loop can start early), then vT
            # (bias on DVE), then pairs 1-3 q/k (bias on DVE, overlapping
            # the start of the attention loop).
            with (
                tc.tile_pool(name="s2", bufs=1) as s2,
                tc.tile_pool(name="s2p", bufs=1, space="PSUM") as s2p,
            ):
                inv_sw = s2.tile([128, 1], f32, name="inv_sw")
                nc.vector.memset(inv_sw[:, :], 1.0 / SW)

                w4q = wq_sb.rearrange("p (j c) -> p j c", j=CT)
                w4k = wk_sb.rearrange("p (j c) -> p j c", j=CT)
                x4 = xt8.rearrange("p (j t) -> p j t", j=CT)
                wv4 = wv_sb.rearrange("p (j c) -> p j c", j=CT)

                def qk_mms(w4, ot):
                    ps = s2p.tile([128, T], f32, name="pq", tag="pq", bufs=3)
                    for kp in range(2):
                        for nt in range(2):
                            nc.tensor.matmul(
                                ps[:, nt * 512 : (nt + 1) * 512],
                                lhsT=w4[:, 2 * kp : 2 * kp + 2,
                                        ot * 128 : (ot + 1) * 128],
                                rhs=x4[:, 2 * kp : 2 * kp + 2,
                                       nt * 512 : (nt + 1) * 512],
                                start=(kp == 0), stop=(kp == 1), perf_mode=DR,
                            )
                    return ps

                def qk_bias_act(ps, b_sb, dst, ot):
                    nc.scalar.activation(
                        out=dst[:, ot * T : (ot + 1) * T],
                        in_=ps[:, :],
                        func=AF.Identity,
                        scale=1.0 / SW,
                        bias=b_sb[:, ot : ot + 1],
                    )

                def qk_bias_dve(ps, b_sb, dst, ot):
                    nc.vector.tensor_scalar(
                        out=dst[:, ot * T : (ot + 1) * T],
                        in0=ps[:, :],
                        scalar1=inv_sw[:, 0:1],
                        scalar2=b_sb[:, ot : ot + 1],
                        op0=AL.mult,
                        op1=AL.add,
                    )

                def vt_emit(tb):
                    # no v bias here: softmax weights sum to 1, so the v
                    # bias rides through the attention average and its
                    # projection image is folded into pb host-side
                    pv2 = s2p.tile([128, 512], f32, name="pv2", tag="pv",
                                   bufs=2)
                    for kp in range(2):
                        nc.tensor.matmul(
                            pv2[:, :],
                            lhsT=x4[:, 2 * kp : 2 * kp + 2,
                                    tb * 128 : (tb + 1) * 128],
                            rhs=wv4[:, 2 * kp : 2 * kp + 2, :],
                            start=(kp == 0), stop=(kp == 1), perf_mode=DR,
                        )
                    # all on ACT (idle pre-loop): keeps the vT matmul
                    # stream off the DVE bias queue entirely
                    nc.scalar.activation(
                        out=vta8[:, tb * C : (tb + 1) * C],
                        in_=pv2[:, :],
                        func=AF.Copy,
                        scale=1.0 / SW,
                    )

                # pair-0 q/k first: bias on ACT, so the attention loop's
                # first scores are only ~2us behind xt8
                ps = qk_mms(w4q, 0)
                qk_bias_act(ps, bq_sb, q16, 0)
                ps = qk_mms(w4k, 0)
                qk_bias_act(ps, bk_sb, k16, 0)
                # pairs 1-3 q/k next: pair-0 scores wait only on the pq
                # rotation, which these biases release quickly
                for ot in range(1, CT):
                    ps = qk_mms(w4q, ot)
                    qk_bias_dve(ps, bq_sb, q16, ot)
                    ps = qk_mms(w4k, ot)
                    qk_bias_dve(ps, bk_sb, k16, ot)
                # vT last: AV consumes vta tb k only at sb k of pair 0,
                # well after the loop starts
                for tb in range(TB):
                    vt_emit(tb)

            # =========== Stage 3: attention ===========
            # PSUM: sc tag bufs=2 (4 banks) + pav (2) + pd (2) = 8 banks.
            # Flat software-pipelined emission: each slot emits its score
            # matmuls FIRST, then pops one deferred AV/d/tail closure from
            # the previous slot — so the PE FIFO never blocks the next
            # exp's input behind AV work that waits on the current exps.
            with (
                tc.tile_pool(name="s3", bufs=1) as s3,
                tc.tile_pool(name="s3p", bufs=1, space="PSUM") as s3p,
            ):
                from collections import deque

                et4 = vta8.rearrange("p (t c) -> p t c", t=TB)
                o2 = ones8_sb.rearrange("p (two o) -> p two o", two=2)
                pending = deque()
                pcur = {}

                def ensure_pav(sbp_or_sb0):
                    if sbp_or_sb0:
                        return
                    pcur["pav"] = s3p.tile(
                        [128, T], f32, name="pav", tag="pav", bufs=1
                    )
                    pcur["pd"] = s3p.tile(
                        [128, T], f32, name="pd", tag="pd", bufs=1
                    )

                def make_b(pr, sb, et8):
                    # B side: non-DR (DR cannot write dst partition base
                    # 64), one sb per matmul — small closure (~4 matmuls)
                    def emit():
                        ensure_pav(sb)
                        pav, pdt = pcur["pav"], pcur["pd"]
                        half = (sb % 2) * 2048
                        lB1 = et4[
                            :, sb, (2 * pr + 1) * CH : (2 * pr + 2) * CH
                        ]
                        stB = dict(start=(sb == 0), stop=(sb == TB - 1))
                        # same-lhsT matmuls adjacent: walrus elides the
                        # repeated Ldweights
                        for nt in range(2):
                            ncol = slice(nt * 512, (nt + 1) * 512)
                            rB = et8[:, half + 1024 + nt * 512 :
                                     half + 1024 + (nt + 1) * 512]
                            nc.tensor.matmul(
                                pav[CH:128, ncol], lhsT=lB1, rhs=rB, **stB
                            )
                        for nt in range(2):
                            ncol = slice(nt * 512, (nt + 1) * 512)
                            rB = et8[:, half + 1024 + nt * 512 :
                                     half + 1024 + (nt + 1) * 512]
                            nc.tensor.matmul(
                                pdt[CH:128, ncol],
                                lhsT=ones8_sb[:, 0:CH],
                                rhs=rB, **stB,
                            )

                    return emit

                def make_a(pr, sbp, et8):
                    def emit():
                        pav, pdt = pcur["pav"], pcur["pd"]
                        e2 = et8.rearrange("p (two c) -> p two c", two=2)
                        lA = et4[
                            :, 2 * sbp : 2 * sbp + 2,
                            2 * pr * CH : (2 * pr + 1) * CH,
                        ]
                        stA = dict(
                            start=(sbp == 0), stop=(sbp == NP - 1),
                            perf_mode=DR,
                        )
                        for nt in range(2):
                            ncol = slice(nt * 512, (nt + 1) * 512)
                            nc.tensor.matmul(
                                pav[0:CH, ncol],
                                lhsT=lA,
                                rhs=e2[:, :, nt * 512 : (nt + 1) * 512],
                                **stA,
                            )
                        # denominator, pre-broadcast over 64 partitions
                        # (ones value 1/SA folds the ar scale in)
                        for nt in range(2):
                            ncol = slice(nt * 512, (nt + 1) * 512)
                            nc.tensor.matmul(
                                pdt[0:CH, ncol],
                                lhsT=o2,
                                rhs=e2[:, :, nt * 512 : (nt + 1) * 512],
                                **stA,
                            )

                    return emit

                def make_tail(pr):
                    def emit():
                        pav, pdt = pcur["pav"], pcur["pd"]
                        rdrep = s3.tile(
                            [128, T], f32, name="rdrep", tag="rd", bufs=2
                        )
                        y0 = s3.tile([128, T], f32, name="y0", tag="y0",
                                     bufs=2)
                        # ~0.4%-accurate reciprocal from standard DVE ops:
                        # BITWISE_NOT seed + Chebyshev-tuned NR pass
                        # (constants from dve_ops.RECIP_APPROX_FAST_CONSTS);
                        # nc.vector.reciprocal is ~6 cycles/elem, too slow.
                        # The last pair runs in column halves so stage 4's
                        # nt0 chain starts after only half the chain.
                        i32 = mybir.dt.int32
                        halves = (
                            (slice(0, 512), slice(512, T))
                            if pr == NP - 1
                            else (slice(0, T),)
                        )
                        for hs in halves:
                            nc.vector.tensor_scalar(
                                out=y0[:, hs].bitcast(i32),
                                in0=pdt[:, hs].bitcast(i32),
                                scalar1=-1, scalar2=None, op0=AL.bitwise_xor,
                            )
                            nc.vector.tensor_scalar(
                                out=y0[:, hs], in0=y0[:, hs],
                                scalar1=-0.23549792, scalar2=None,
                                op0=AL.mult,
                            )
                            nc.vector.tensor_tensor(
                                out=rdrep[:, hs], in0=pdt[:, hs],
                                in1=y0[:, hs], op=AL.mult,
                            )
                            nc.vector.tensor_scalar(
                                out=rdrep[:, hs], in0=rdrep[:, hs],
                                scalar1=-1.0, scalar2=2.0017324,
                                op0=AL.mult, op1=AL.add,
                            )
                            nc.vector.tensor_tensor(
                                out=rdrep[:, hs], in0=rdrep[:, hs],
                                in1=y0[:, hs], op=AL.mult,
                            )
                            nc.vector.tensor_tensor(
                                out=ar8[:, pr * T : (pr + 1) * T][:, hs],
                                in0=pav[:, hs],
                                in1=rdrep[:, hs],
                                op=AL.mult,
                            )
                        if debug:
                            pdc = s3.tile(
                                [128, T], f32, name="pdc", tag="pdc", bufs=2
                            )
                            nc.vector.tensor_copy(out=pdc[:, :], in_=pdt[:, :])
                            nc.sync.dma_start(
                                out=dbg_pd[pr * 128 : (pr + 1) * 128, :],
                                in_=pdc[:, :],
                            )

                    return emit

                two_c = s3.tile([128, 1], f32, name="two_c")
                nc.vector.memset(two_c[:, :], 2.0)

                def dve_exp(sc, et_out):
                    # exp via degree-3 polynomial on s/4 then two squarings
                    # (all standard DVE ops) — offloads the ACT wall.
                    # p3(x) = (1/6)((x^2+2x+4)(x+1)+2) = 1+x+x^2/2+x^3/6;
                    # exp(s) ~ p3(s/4)^4, rel err <~1% for |s|<2.
                    px = s3.tile([128, T], f32, name="px", tag="px", bufs=2)
                    py = s3.tile([128, T], f32, name="py", tag="py", bufs=2)
                    nc.vector.tensor_scalar(
                        out=px[:, :], in0=sc[:, :],
                        scalar1=0.25, scalar2=None, op0=AL.mult,
                    )
                    nc.vector.tensor_scalar(
                        out=py[:, :], in0=px[:, :],
                        scalar1=1.0, scalar2=None, op0=AL.add,
                    )
                    nc.vector.scalar_tensor_tensor(
                        out=px[:, :], in0=px[:, :], scalar=two_c[:, 0:1],
                        in1=px[:, :], op0=AL.add, op1=AL.mult,
                    )
                    nc.vector.tensor_scalar(
                        out=px[:, :], in0=px[:, :],
                        scalar1=4.0, scalar2=None, op0=AL.add,
                    )
                    nc.vector.tensor_tensor(
                        out=py[:, :], in0=px[:, :], in1=py[:, :], op=AL.mult
                    )
                    nc.vector.tensor_scalar(
                        out=py[:, :], in0=py[:, :],
                        scalar1=1.0 / 6.0, scalar2=1.0 / 3.0,
                        op0=AL.mult, op1=AL.add,
                    )
                    nc.vector.tensor_tensor(
                        out=py[:, :], in0=py[:, :], in1=py[:, :], op=AL.mult
                    )
                    nc.vector.tensor_tensor(
                        out=et_out, in0=py[:, :], in1=py[:, :], op=AL.mult
                    )

                et8 = None
                b_del = None
                delayed = []
                OFF_SB, OFF_NT = 4, 1  # one DVE-offloaded exp tile per pair
                for gsb in range(NH * TB // 2):
                    pr, sb = divmod(gsb, TB)
                    for item in [d for d in delayed if d[0] <= gsb]:
                        pending.append(item[1])
                        delayed.remove(item)
                    if b_del is not None:
                        pending.append(b_del)
                        b_del = None
                    qA = q16[0:CH, pr * T : (pr + 1) * T]
                    kA = k16[0:CH, pr * T : (pr + 1) * T]
                    qB = q16[CH:128, pr * T : (pr + 1) * T]
                    kB = k16[CH:128, pr * T : (pr + 1) * T]
                    if sb % 2 == 0:
                        et8 = s3.tile(
                            [128, 4096], fp8, name="et8", tag="et", bufs=3
                        )
                    for nt in range(2):
                        sc = s3p.tile([128, T], f32, name="sc", tag="sc",
                                      bufs=2)
                        nc.tensor.matmul(
                            sc[:, 0:512],
                            lhsT=kA[:, sb * 128 : (sb + 1) * 128],
                            rhs=qA[:, nt * 512 : (nt + 1) * 512],
                            start=True, stop=True,
                        )
                        nc.tensor.matmul(
                            sc[:, 512:1024],
                            lhsT=kB[:, sb * 128 : (sb + 1) * 128],
                            rhs=qB[:, nt * 512 : (nt + 1) * 512],
                            start=True, stop=True,
                        )
                        # one small deferred closure per nt slot keeps the
                        # PE bursts short so ACT never starves
                        if pending:
                            pending.popleft()()
                        # out: A-ntX -> et8[half + nt*512],
                        #      B-ntX -> et8[half + 1024 + nt*512]
                        et_out = et8[:, :].rearrange(
                            "p (h ab n) -> p h ab n", h=2, ab=2
                        )[:, sb % 2, :, nt * 512 : (nt + 1) * 512]
                        if sb == OFF_SB and nt == OFF_NT:
                            dve_exp(sc, et_out)
                        else:
                            nc.scalar.activation(
                                out=et_out, in_=sc[:, :], func=AF.Exp
                            )
                    if sb == OFF_SB:
                        # consumers of the DVE-produced tile wait one extra
                        # slot so the PE FIFO never blocks on the chain
                        delayed.append((gsb + 2, make_b(pr, sb, et8)))
                    else:
                        b_del = make_b(pr, sb, et8)
                    if sb % 2 == 1:
                        mk_a = make_a(pr, sb // 2, et8)
                        if sb // 2 == OFF_SB // 2:
                            delayed.append((gsb + 2, mk_a))
                        elif sb == TB - 1:
                            # the tail's reciprocal reads pd: the last B
                            # contribution must be emitted before it
                            pending.append(b_del)
                            b_del = None
                            pending.append(mk_a)
                            pending.append(make_tail(pr))
                        else:
                            pending.append(mk_a)
                        if sb == TB - 1 and sb // 2 == OFF_SB // 2:
                            pending.append(b_del)
                            b_del = None
                            pending.append(make_tail(pr))
                if b_del is not None:
                    pending.append(b_del)
                for _, cl in delayed:
                    pending.append(cl)
                while pending:
                    pending.popleft()()

            # =========== Stage 4: proj + residual ===========
            with (
                tc.tile_pool(name="s4", bufs=1) as s4,
                tc.tile_pool(name="s4p", bufs=1, space="PSUM") as s4p,
            ):
                inv_o = s4.tile([128, 1], f32, name="inv_o")
                nc.vector.memset(inv_o[:, :], 1.0 / (SW * SA))
                # xpb = xt + proj bias: emitted here so the DVE ops run in
                # loop slack, well before the stts below need them
                for j in range(CT):
                    nc.vector.tensor_scalar(
                        out=xpb[:, j * T : (j + 1) * T],
                        in0=xt[:, j * T : (j + 1) * T],
                        scalar1=pb_sb[:, j : j + 1],
                        scalar2=None,
                        op0=AL.add,
                    )
                pw4 = pw_sb.rearrange("p (j c) -> p j c", j=CT)
                ar4 = ar8.rearrange("p (j t) -> p j t", j=CT)
                for nt in range(2):
                    # nt-major: the nt0 chain starts as soon as the last
                    # pair's first normalization half lands in ar8
                    for j in range(CT):
                        po = s4p.tile(
                            [128, 512], f32, name="po", tag="po", bufs=4
                        )
                        for kp in range(2):
                            nc.tensor.matmul(
                                po[:, :],
                                lhsT=pw4[:, 2 * kp : 2 * kp + 2,
                                         j * 128 : (j + 1) * 128],
                                rhs=ar4[:, 2 * kp : 2 * kp + 2,
                                        nt * 512 : (nt + 1) * 512],
                                start=(kp == 0), stop=(kp == 1), perf_mode=DR,
                            )
                        ot_ = s4.tile([128, 512], f32, name="ot_", tag="ot",
                                      bufs=4)
                        xpb_sl = xpb[:, j * T + nt * 512 :
                                     j * T + nt * 512 + 512]
                        if j % 2 == 0:
                            nc.vector.scalar_tensor_tensor(
                                out=ot_[:, :],
                                in0=po[:, :],
                                scalar=inv_o[:, 0:1],
                                in1=xpb_sl,
                                op0=AL.mult,
                                op1=AL.add,
                            )
                        else:
                            # tail is DVE-serial otherwise: route half the
                            # chunks through idle ACT (scale) + GPSIMD (add)
                            pc = s4.tile([128, 512], f32, name="pc", tag="pc",
                                         bufs=2)
                            nc.scalar.activation(
                                out=pc[:, :], in_=po[:, :], func=AF.Copy,
                                scale=1.0 / (SW * SA),
                            )
                            nc.gpsimd.tensor_tensor(
                                out=ot_[:, :], in0=pc[:, :], in1=xpb_sl,
                                op=AL.add,
                            )
                        # loop mode: y owns the scalar queue (drains into
                        # the next iteration); single-shot spreads over 3
                        if loop_n:
                            eng = dma2
                        else:
                            eng = (nc.sync, dma2, nc.gpsimd, nc.sync)[j]
                        eng.dma_start(
                            out=y[j * 128 : (j + 1) * 128,
                                  nt * 512 : (nt + 1) * 512],
                            in_=ot_[:, :],
                        )

            if debug:
                for j in range(CT):
                    sl = slice(j * 128, (j + 1) * 128)
                    ts = slice(j * T, (j + 1) * T)
                    nc.sync.dma_start(
                        out=dbg_xt8[sl, :], in_=xt8[:, ts].bitcast(u8)
                    )
                    nc.sync.dma_start(out=dbg_q[sl, :], in_=q16[:, ts])
                    nc.sync.dma_start(out=dbg_k[sl, :], in_=k16[:, ts])
                    nc.sync.dma_start(
                        out=dbg_ar[sl, :], in_=ar8[:, ts].bitcast(u8)
                    )
                nc.sync.dma_start(out=dbg_v[:, :], in_=vta8[:, :].bitcast(u8))

            loop_cm.__exit__(None, None, None)

    return nc


def _prep_host(norm_scale, norm_bias, qkv_w, qkv_b, proj_w, proj_b):
    import ml_dtypes

    f8 = ml_dtypes.float8_e4m3
    s = float(CH) ** -0.25
    w3 = qkv_w.reshape(NH, 3, CH, C)
    b3 = qkv_b.reshape(NH, 3, CH)

    def to8(a):
        return np.ascontiguousarray(a).astype(f8).view(np.uint8)

    wq = to8((w3[:, 0] * s).reshape(C, C).T * SW)
    wk = to8((w3[:, 1] * s).reshape(C, C).T * SW)
    wv = to8(w3[:, 2].reshape(C, C).T * SW)
    pw = to8(proj_w.T * SW)
    # all biases are added AFTER the 1/SW rescale, so they stay unscaled.
    # v bias: softmax weights sum to 1, so its contribution to the
    # attention output is the constant vector bv, folded into pb here.
    bq_ = np.ascontiguousarray((b3[:, 0] * s).reshape(C))
    bk_ = np.ascontiguousarray((b3[:, 1] * s).reshape(C))
    bv_ = np.ascontiguousarray(b3[:, 2].reshape(C))
    pb_ = np.ascontiguousarray(proj_b + proj_w @ bv_)
    c = np.arange(C)
    gmap_ = (c[:, None] // GS == np.arange(NG)[None, :]).astype(np.float32)
    emap_ = np.ascontiguousarray(gmap_.T)
    ones8_ = np.full((128, 128), 1.0 / SA, dtype=np.float32).astype(f8).view(
        np.uint8
    )
    return {
        "wq8": wq, "wk8": wk, "wv8": wv, "pw8": pw,
        "bq": bq_.astype(np.float32), "bk": bk_.astype(np.float32),
        "pb": pb_.astype(np.float32),
        "nsc": norm_scale.astype(np.float32),
        "nbi": norm_bias.astype(np.float32),
        "gmap": gmap_, "emap": emap_, "ones8": ones8_,
    }


def make_in_maps(x, norm_scale, norm_bias, qkv_w, qkv_b, proj_w, proj_b):
    shared = _prep_host(norm_scale, norm_bias, qkv_w, qkv_b, proj_w, proj_b)
    in_maps = []
    for b in range(N_CORES):
        m = dict(shared)
        m["x"] = np.ascontiguousarray(x[b].reshape(C, T).astype(np.float32))
        in_maps.append(m)
    return in_maps


def get_nc(split_waits=True, loop_n=None):
    key = ("nc2", split_waits, loop_n)
    if key not in _CACHE:
        from concourse import mybir
        import bass_rust

        nc = build_nc(loop_n=loop_n)
        if split_waits:
            _split_excess_waits(nc, mybir, bass_rust)
        _CACHE[key] = nc
    return _CACHE[key]


def kernel(x, norm_scale, norm_bias, qkv_w, qkv_b, proj_w, proj_b):
    from concourse.bass_utils import run_bass_kernel_spmd

    # NOTE: no LDW-opt patch here — the walrus LDW optimization rejects
    # bf16/fp8 Ldweights, and this kernel's matmuls are all bf16/fp8.
    nc = get_nc()
    in_maps = make_in_maps(
        x, norm_scale, norm_bias, qkv_w, qkv_b, proj_w, proj_b
    )
    res = run_bass_kernel_spmd(nc, in_maps, core_ids=list(range(N_CORES)))
    out = np.stack([res.results[b]["y"] for b in range(N_CORES)], axis=0)
    return out.reshape(B, C, 32, 32).astype(np.float32)
